# revision 9
# baseline (speedup 1.0000x reference)
"""Causal multi-head self-attention with RoPE on 8 TRN2 NeuronCores.

Sharding: batch(4) x head-group(2) -> 8 cores. Core c handles batch c//2 and
heads [8*(c%2), 8*(c%2)+8). Each core computes its partial output projection
(sum over its 8 heads' contribution); the host adds the two head-group
partials per batch. No device collectives needed.

On-chip layout: sequence lives on the free dimension everywhere.
  - Q^T/K^T [j, s] come straight out of the projection matmuls
    (lhsT = W^T slices, rhs = x^T), RoPE applied with a partition pair-swap
    (stream_shuffle) + precomputed cos/sin tables.
  - scores are computed transposed S^T = K^T.T-contraction -> [k, q] tiles,
    exp on ScalarE (no max subtraction needed: |scores| <= ~15), causal
    masking by adding -1e30 on diagonal tiles before exp.
  - P@V contraction runs over k on partitions; a ones-row appended to V
    makes the softmax denominator fall out of the same matmul (M=65).
  - output projection contracts the 512 head-dims -> partial y^T [1024, s].
"""

import os
import sys
import time

for _p in ("/opt/trn_rl_repo", "/root/.axon_site/_ro/trn_rl_repo"):
    if _p not in sys.path and os.path.isdir(_p):
        sys.path.insert(0, _p)

import numpy as np
import concourse.bass as bass
import concourse.bacc as bacc
import concourse.mybir as mybir
import concourse.tile as tile
from concourse.bass_utils import run_bass_kernel_spmd

F32 = mybir.dt.float32
F32R = mybir.dt.float32r

B, S, D = 4, 2048, 1024
H, DK = 16, 64
HPC = 8            # heads per core
JC = HPC * DK      # 512 head-dims per core
N_CORES = 8
SC = 512           # q-chunk width (moving free dim)
NSC = S // SC      # 4
KT = 128           # k-tile (scores partition dim)
NKT = S // KT      # 16
DT = D // 128      # 8 contraction tiles for projections

# matmul operand dtype: "f32" (exact) or "f32r" (tf32-like, ~4x faster PE)
MM_DTYPE = os.environ.get("KERNEL_MM_DTYPE", "f32")

_PAIR_SWAP = []
for _i in range(16):
    _PAIR_SWAP += [2 * _i + 1, 2 * _i]


def _emit(nc, tc, mmdt, dram, tag=""):
    """Emit the whole per-core program. `dram` maps name -> DRAM AP."""
    xT = dram["xT"]
    wq, wk, wv, wo = dram["wq"], dram["wk"], dram["wv"], dram["wo"]
    cosE, sinE, maskneg = dram["cosE"], dram["sinE"], dram["maskneg"]
    yT = dram["yT"]

    need_round = mmdt != F32
    EXP = mybir.ActivationFunctionType.Exp

    import contextlib
    with contextlib.ExitStack() as ctx:
        # ---- persistent tiles -------------------------------------------
        per = ctx.enter_context(tc.tile_pool(name=f"per{tag}", bufs=1))
        QT = [per.tile([128, S], mmdt, tag=f"QT{j}{tag}", name=f"QT{j}{tag}") for j in range(4)]
        KTt = [per.tile([128, S], mmdt, tag=f"KT{j}{tag}", name=f"KT{j}{tag}") for j in range(4)]
        vo = [per.tile([128, HPC, 65], mmdt, tag=f"vo{i}{tag}", name=f"vo{i}{tag}") for i in range(NKT)]
        cos_sb = per.tile([128, S], F32, tag=f"cos{tag}", name=f"cos{tag}")
        sin_sb = per.tile([128, S], F32, tag=f"sin{tag}", name=f"sin{tag}")
        mask_sb = per.tile([128, 4, SC], F32, tag=f"mask{tag}", name=f"mask{tag}")
        nc.sync.dma_start(out=cos_sb, in_=cosE)
        nc.sync.dma_start(out=sin_sb, in_=sinE)
        nc.sync.dma_start(out=mask_sb, in_=maskneg.rearrange("p (d q) -> p d q", d=4))

        # ---- phase A: projections ---------------------------------------
        with tc.tile_pool(name=f"pA{tag}", bufs=1) as pa, \
             tc.tile_pool(name=f"pAx{tag}", bufs=1) as pax, \
             tc.tile_pool(name=f"pAt{tag}", bufs=1) as pat, \
             tc.tile_pool(name=f"pAps{tag}", bufs=2, space="PSUM") as paps:
            w_sb = {}
            for name, w_ap in (("wq", wq), ("wk", wk), ("wv", wv)):
                if need_round:
                    stage = pax.tile([128, DT, JC], F32, tag=f"stg{tag}", name=f"stg{tag}")
                    nc.sync.dma_start(
                        out=stage, in_=w_ap.rearrange("(dt p) j -> p dt j", p=128))
                    wt = pa.tile([128, DT, JC], mmdt, tag=f"{name}{tag}", name=f"{name}{tag}")
                    nc.vector.tensor_copy(wt, stage)
                else:
                    wt = pa.tile([128, DT, JC], mmdt, tag=f"{name}{tag}", name=f"{name}{tag}")
                    nc.sync.dma_start(
                        out=wt, in_=w_ap.rearrange("(dt p) j -> p dt j", p=128))
                w_sb[name] = wt

            xT_r = xT.rearrange("(dt p) s -> p dt s", p=128)
            for sc in range(NSC):
                ssl = slice(sc * SC, (sc + 1) * SC)
                if need_round:
                    xstage = pax.tile([128, DT, SC], F32, tag=f"stg{tag}", name=f"stg{tag}")
                    nc.sync.dma_start(out=xstage, in_=xT_r[:, :, ssl])
                    xc = pax.tile([128, DT, SC], mmdt, tag=f"xc{tag}", name=f"xc{tag}")
                    nc.vector.tensor_copy(xc, xstage)
                else:
                    xc = pax.tile([128, DT, SC], mmdt, tag=f"xc{tag}",
                                  name=f"xc{tag}", bufs=2)
                    nc.sync.dma_start(out=xc, in_=xT_r[:, :, ssl])

                # V: out[s, j] tiles, lhsT = x^T [d, s], rhs = Wv^T [d, j]
                for st in range(4):
                    pv = paps.tile([128, JC], F32, tag=f"vps{tag}", name=f"vps{tag}")
                    sl = slice(st * 128, (st + 1) * 128)
                    for dt in range(DT):
                        nc.tensor.matmul(
                            pv, xc[:, dt, sl], w_sb["wv"][:, dt, :],
                            start=(dt == 0), stop=(dt == DT - 1))
                    vt = vo[sc * 4 + st]
                    nc.vector.tensor_copy(
                        vt[:, :, 0:64],
                        pv.rearrange("p (h j) -> p h j", h=HPC))
                    nc.vector.memset(vt[:, :, 64:65], 1.0)

                # Q/K: out = (W row-slice) @ x^T -> [j, s] + RoPE
                for wname, dst in (("wq", QT), ("wk", KTt)):
                    for jt in range(4):
                        ps = paps.tile([128, SC], F32, tag=f"qkps{tag}", name=f"qkps{tag}")
                        jl = slice(jt * 128, (jt + 1) * 128)
                        for dt in range(DT):
                            nc.tensor.matmul(
                                ps, w_sb[wname][:, dt, jl], xc[:, dt, :],
                                start=(dt == 0), stop=(dt == DT - 1))
                        qs = pat.tile([128, SC], F32, tag=f"ropes{tag}", name=f"ropes{tag}")
                        nc.vector.stream_shuffle(qs, ps, _PAIR_SWAP)
                        qc_t = pat.tile([128, SC], F32, tag=f"ropec{tag}", name=f"ropec{tag}")
                        nc.vector.tensor_mul(qc_t, ps, cos_sb[:, ssl])
                        nc.vector.tensor_mul(qs, qs, sin_sb[:, ssl])
                        nc.vector.tensor_add(dst[jt][:, ssl], qc_t, qs)

        # ---- phase B: attention + output projection ---------------------
        with tc.tile_pool(name=f"pB{tag}", bufs=1) as pb, \
             tc.tile_pool(name=f"pBe{tag}", bufs=3) as pbe, \
             tc.tile_pool(name=f"pBt{tag}", bufs=2) as pbt, \
             tc.tile_pool(name=f"pBo{tag}", bufs=2) as pbo, \
             tc.tile_pool(name=f"pBps{tag}", bufs=1, space="PSUM") as pbps, \
             tc.tile_pool(name=f"pBps2{tag}", bufs=2, space="PSUM") as pbps2:
            if need_round:
                wostg = pbt.tile([128, 4, D], F32, tag=f"wostg{tag}", name=f"wostg{tag}")
                nc.sync.dma_start(
                    out=wostg, in_=wo.rearrange("(hp p) m -> p hp m", p=128))
                wo_sb = pb.tile([128, 4, D], mmdt, tag=f"wo{tag}", name=f"wo{tag}")
                nc.vector.tensor_copy(wo_sb, wostg)
            else:
                wo_sb = pb.tile([128, 4, D], mmdt, tag=f"wo{tag}", name=f"wo{tag}")
                nc.sync.dma_start(
                    out=wo_sb, in_=wo.rearrange("(hp p) m -> p hp m", p=128))

            for qc in range(NSC):
                qsl = slice(qc * SC, (qc + 1) * SC)
                oTs = []
                for hp in range(4):
                    pva = pbps.tile([65, SC], F32, tag=f"pva{tag}", name=f"pva{tag}")
                    pvb = pbps.tile([65, SC], F32, tag=f"pvb{tag}", name=f"pvb{tag}")
                    nkt = 4 * qc + 4
                    for kt in range(nkt):
                        ksl = slice(kt * KT, (kt + 1) * KT)
                        sca = pbps2.tile([128, SC], F32, tag=f"sca{tag}", name=f"sca{tag}")
                        scb = pbps2.tile([128, SC], F32, tag=f"scb{tag}", name=f"scb{tag}")
                        nc.tensor.matmul(sca, KTt[hp][0:64, ksl],
                                         QT[hp][0:64, qsl],
                                         start=True, stop=True,
                                         tile_position=(0, 0))
                        nc.tensor.matmul(scb, KTt[hp][64:128, ksl],
                                         QT[hp][64:128, qsl],
                                         start=True, stop=True,
                                         tile_position=(64, 0))
                        d = kt - 4 * qc
                        if d >= 0:  # diagonal tile: causal mask pre-exp
                            nc.vector.tensor_add(sca, sca, mask_sb[:, d, :])
                            nc.vector.tensor_add(scb, scb, mask_sb[:, d, :])
                        ea = pbe.tile([128, SC], mmdt, tag=f"ea{tag}", name=f"ea{tag}")
                        eb = pbe.tile([128, SC], mmdt, tag=f"eb{tag}", name=f"eb{tag}")
                        nc.scalar.activation(ea, sca, EXP, scale=0.125)
                        nc.scalar.activation(eb, scb, EXP, scale=0.125)
                        h0, h1 = 2 * hp, 2 * hp + 1
                        nc.tensor.matmul(
                            pva, vo[kt][:, h0, :], ea,
                            start=(kt == 0), stop=(kt == nkt - 1))
                        nc.tensor.matmul(
                            pvb, vo[kt][:, h1, :], eb,
                            start=(kt == 0), stop=(kt == nkt - 1))
                    # normalize: oT[j, q] = pv[j, q] / denom[q].
                    # All DVE ops must be partition-aligned; the denom row
                    # lives at partition 64, so recip in place (64->64), DMA
                    # the row to partition 0, broadcast to 0:64, multiply at
                    # base 0, and DMA-relocate head B's rows to 64:128.
                    rcA = pbt.tile([65, SC], F32, tag=f"rcA{tag}", name=f"rcA{tag}")
                    rcB = pbt.tile([65, SC], F32, tag=f"rcB{tag}", name=f"rcB{tag}")
                    nc.vector.reciprocal(rcA[64:65, :], pva[64:65, :])
                    nc.vector.reciprocal(rcB[64:65, :], pvb[64:65, :])
                    rA0 = pbt.tile([1, SC], F32, tag=f"rA0{tag}", name=f"rA0{tag}")
                    rB0 = pbt.tile([1, SC], F32, tag=f"rB0{tag}", name=f"rB0{tag}")
                    nc.sync.dma_start(out=rA0, in_=rcA[64:65, :])
                    nc.sync.dma_start(out=rB0, in_=rcB[64:65, :])
                    bcA = pbt.tile([64, SC], F32, tag=f"bcA{tag}", name=f"bcA{tag}")
                    bcB = pbt.tile([64, SC], F32, tag=f"bcB{tag}", name=f"bcB{tag}")
                    nc.gpsimd.partition_broadcast(bcA, rA0)
                    nc.gpsimd.partition_broadcast(bcB, rB0)
                    oT = pbo.tile([128, SC], mmdt, tag=f"oT{hp}{tag}", name=f"oT{hp}{tag}")
                    tmpB = pbt.tile([64, SC], mmdt, tag=f"tmpB{tag}", name=f"tmpB{tag}")
                    nc.vector.tensor_mul(oT[0:64, :], pva[0:64, :], bcA)
                    nc.vector.tensor_mul(tmpB, pvb[0:64, :], bcB)
                    nc.sync.dma_start(out=oT[64:128, :], in_=tmpB)
                    oTs.append(oT)

                for mt in range(8):
                    yps = pbps2.tile([128, SC], F32, tag=f"yps{tag}", name=f"yps{tag}")
                    ml = slice(mt * 128, (mt + 1) * 128)
                    for hp in range(4):
                        nc.tensor.matmul(yps, wo_sb[:, hp, ml], oTs[hp],
                                         start=(hp == 0), stop=(hp == 3))
                    ys = pbt.tile([128, SC], F32, tag=f"ys{tag}", name=f"ys{tag}")
                    nc.vector.tensor_copy(ys, yps)
                    nc.sync.dma_start(out=yT[ml, qsl], in_=ys)


_BUILT = {}


def build_nc(mmdt_name=MM_DTYPE, repeat=1):
    key = (mmdt_name, repeat)
    if key in _BUILT:
        return _BUILT[key]
    mmdt = {"f32": F32, "f32r": F32R}[mmdt_name]
    nc = bacc.Bacc("TRN2", target_bir_lowering=False, debug=False,
                   num_devices=N_CORES)
    dram = {
        "xT": nc.dram_tensor("xT", [D, S], F32, kind="ExternalInput").ap(),
        "wq": nc.dram_tensor("wq", [D, JC], F32, kind="ExternalInput").ap(),
        "wk": nc.dram_tensor("wk", [D, JC], F32, kind="ExternalInput").ap(),
        "wv": nc.dram_tensor("wv", [D, JC], F32, kind="ExternalInput").ap(),
        "wo": nc.dram_tensor("wo", [JC, D], F32, kind="ExternalInput").ap(),
        "cosE": nc.dram_tensor("cosE", [128, S], F32,
                               kind="ExternalInput").ap(),
        "sinE": nc.dram_tensor("sinE", [128, S], F32,
                               kind="ExternalInput").ap(),
        "maskneg": nc.dram_tensor("maskneg", [128, 4 * SC], F32,
                                  kind="ExternalInput").ap(),
        "yT": nc.dram_tensor("yT", [D, S], F32, kind="ExternalOutput").ap(),
    }
    with tile.TileContext(nc) as tc:
        for r in range(repeat):
            _emit(nc, tc, mmdt, dram, tag=f"r{r}" if repeat > 1 else "")
    nc.compile()
    _BUILT[key] = nc
    return nc


def _host_prep(x, pos_ids, Wq, Wk, Wv, Wo, cos, sin):
    """Build the 8 per-core input maps."""
    x = np.asarray(x, dtype=np.float32)
    pos_ids = np.asarray(pos_ids)
    cos = np.asarray(cos, dtype=np.float32)
    sin = np.asarray(sin, dtype=np.float32)
    freq_idx = np.tile(np.repeat(np.arange(DK // 2), 2), 2)  # [128]
    sign = np.where((np.arange(128) % 2) == 0, -1.0, 1.0).astype(np.float32)

    # causal mask tiles: mask[d][p, q] = 0 if q >= 128*d + p else -1e30
    p = np.arange(128)[:, None]
    q = np.arange(SC)[None, :]
    mask = np.concatenate(
        [np.where(q >= 128 * d + p, 0.0, -1e30).astype(np.float32)
         for d in range(4)], axis=1)  # [128, 2048]

    in_maps = []
    for c in range(N_CORES):
        b, g = c // 2, c % 2
        hs = slice(64 * HPC * g, 64 * HPC * g + JC)
        pos = pos_ids[b].astype(np.int64)
        cosT = cos[pos].T  # [32, S]
        sinT = sin[pos].T
        cosE = np.ascontiguousarray(cosT[freq_idx])           # [128, S]
        sinE = np.ascontiguousarray(sinT[freq_idx] * sign[:, None])
        in_maps.append({
            "xT": np.ascontiguousarray(x[b].T),
            "wq": np.ascontiguousarray(Wq[hs, :].T),
            "wk": np.ascontiguousarray(Wk[hs, :].T),
            "wv": np.ascontiguousarray(Wv[hs, :].T),
            "wo": np.ascontiguousarray(Wo[:, hs].T),
            "cosE": cosE,
            "sinE": sinE,
            "maskneg": mask,
        })
    return in_maps


def kernel(x, pos_ids, Wq, Wk, Wv, Wo, cos, sin):
    nc = build_nc()
    in_maps = _host_prep(x, pos_ids, Wq, Wk, Wv, Wo, cos, sin)
    res = run_bass_kernel_spmd(nc, in_maps, list(range(N_CORES)))
    out = np.empty((B, S, D), dtype=np.float32)
    for b in range(B):
        yT = res.results[2 * b]["yT"] + res.results[2 * b + 1]["yT"]
        out[b] = yT.T
    return out


if __name__ == "__main__":
    t0 = time.time()
    nc = build_nc()
    print(f"build+compile: {time.time()-t0:.1f}s", flush=True)


# revision 14
# speedup vs baseline: 2.3163x; 2.3163x over previous
"""Causal multi-head self-attention with RoPE on 8 TRN2 NeuronCores.

Sharding: batch(4) x head-group(2) -> 8 cores. Core c handles batch c//2 and
heads [8*(c%2), 8*(c%2)+8). Each core computes its partial output projection
(sum over its 8 heads' contribution); the host adds the two head-group
partials per batch. No device collectives needed.

On-chip layout: sequence lives on the free dimension everywhere.
  - Q^T/K^T [j, s] come straight out of the projection matmuls
    (lhsT = W^T slices, rhs = x^T), RoPE applied with a partition pair-swap
    (stream_shuffle) + precomputed cos/sin tables.
  - scores are computed transposed S^T = K^T.T-contraction -> [k, q] tiles,
    exp on ScalarE (no max subtraction needed: |scores| <= ~15), causal
    masking by adding -1e30 on diagonal tiles before exp.
  - P@V contraction runs over k on partitions; a ones-row appended to V
    makes the softmax denominator fall out of the same matmul (M=65).
  - output projection contracts the 512 head-dims -> partial y^T [1024, s].
"""

import os
import sys
import time

for _p in ("/opt/trn_rl_repo", "/root/.axon_site/_ro/trn_rl_repo"):
    if _p not in sys.path and os.path.isdir(_p):
        sys.path.insert(0, _p)

import numpy as np
import concourse.bass as bass
import concourse.bacc as bacc
import concourse.mybir as mybir
import concourse.tile as tile
from concourse.bass_utils import run_bass_kernel_spmd

F32 = mybir.dt.float32
F32R = mybir.dt.float32r

B, S, D = 4, 2048, 1024
H, DK = 16, 64
HPC = 8            # heads per core
JC = HPC * DK      # 512 head-dims per core
N_CORES = 8
SC = 512           # q-chunk width (moving free dim)
NSC = S // SC      # 4
KT = 128           # k-tile (scores partition dim)
NKT = S // KT      # 16
DT = D // 128      # 8 contraction tiles for projections

# matmul operand dtype: "f32" (exact) or "f32r" (tf32-like, ~4x faster PE)
MM_DTYPE = os.environ.get("KERNEL_MM_DTYPE", "f32")

_PAIR_SWAP = []
for _i in range(16):
    _PAIR_SWAP += [2 * _i + 1, 2 * _i]


def _emit(nc, tc, mmdt, dram, tag=""):
    """Emit the whole per-core program. `dram` maps name -> DRAM AP."""
    xT = dram["xT"]
    wq, wk, wv, wo = dram["wq"], dram["wk"], dram["wv"], dram["wo"]
    cosE, sinE, maskneg = dram["cosE"], dram["sinE"], dram["maskneg"]
    yT = dram["yT"]

    need_round = mmdt != F32
    EXP = mybir.ActivationFunctionType.Exp

    import contextlib
    with contextlib.ExitStack() as ctx:
        # ---- persistent tiles -------------------------------------------
        per = ctx.enter_context(tc.tile_pool(name=f"per{tag}", bufs=1))
        QT = [per.tile([128, S], mmdt, tag=f"QT{j}{tag}", name=f"QT{j}{tag}") for j in range(4)]
        KTt = [per.tile([128, S], mmdt, tag=f"KT{j}{tag}", name=f"KT{j}{tag}") for j in range(4)]
        vo = [per.tile([128, HPC, 65], mmdt, tag=f"vo{i}{tag}", name=f"vo{i}{tag}") for i in range(NKT)]
        ones_sb = per.tile([128, HPC], F32, tag=f"ones{tag}", name=f"ones{tag}")
        nc.vector.memset(ones_sb, 1.0)
        cos_sb = per.tile([128, S], F32, tag=f"cos{tag}", name=f"cos{tag}")
        sin_sb = per.tile([128, S], F32, tag=f"sin{tag}", name=f"sin{tag}")
        mask_sb = per.tile([128, 4, SC], F32, tag=f"mask{tag}", name=f"mask{tag}")
        nc.sync.dma_start(out=cos_sb, in_=cosE)
        nc.sync.dma_start(out=sin_sb, in_=sinE)
        nc.sync.dma_start(out=mask_sb, in_=maskneg.rearrange("p (d q) -> p d q", d=4))

        # ---- phase A: projections ---------------------------------------
        with tc.tile_pool(name=f"pA{tag}", bufs=1) as pa, \
             tc.tile_pool(name=f"pAx{tag}", bufs=1) as pax, \
             tc.tile_pool(name=f"pAt{tag}", bufs=1) as pat, \
             tc.tile_pool(name=f"pAps{tag}", bufs=2, space="PSUM") as paps:
            w_sb = {}
            for name, w_ap in (("wq", wq), ("wk", wk), ("wv", wv)):
                if need_round:
                    stage = pax.tile([128, DT, JC], F32, tag=f"stg{tag}", name=f"stg{tag}")
                    nc.sync.dma_start(
                        out=stage, in_=w_ap.rearrange("(dt p) j -> p dt j", p=128))
                    wt = pa.tile([128, DT, JC], mmdt, tag=f"{name}{tag}", name=f"{name}{tag}")
                    nc.vector.tensor_copy(wt, stage)
                else:
                    wt = pa.tile([128, DT, JC], mmdt, tag=f"{name}{tag}", name=f"{name}{tag}")
                    nc.sync.dma_start(
                        out=wt, in_=w_ap.rearrange("(dt p) j -> p dt j", p=128))
                w_sb[name] = wt

            xT_r = xT.rearrange("(dt p) s -> p dt s", p=128)
            for sc in range(NSC):
                ssl = slice(sc * SC, (sc + 1) * SC)
                if need_round:
                    xstage = pax.tile([128, DT, SC], F32, tag=f"stg{tag}", name=f"stg{tag}")
                    nc.sync.dma_start(out=xstage, in_=xT_r[:, :, ssl])
                    xc = pax.tile([128, DT, SC], mmdt, tag=f"xc{tag}", name=f"xc{tag}")
                    nc.vector.tensor_copy(xc, xstage)
                else:
                    xc = pax.tile([128, DT, SC], mmdt, tag=f"xc{tag}",
                                  name=f"xc{tag}", bufs=2)
                    nc.sync.dma_start(out=xc, in_=xT_r[:, :, ssl])

                # V: out[s, j] tiles, lhsT = x^T [d, s], rhs = Wv^T [d, j]
                for st in range(4):
                    pv = paps.tile([128, JC], F32, tag=f"vps{tag}", name=f"vps{tag}")
                    sl = slice(st * 128, (st + 1) * 128)
                    for dt in range(DT):
                        nc.tensor.matmul(
                            pv, xc[:, dt, sl], w_sb["wv"][:, dt, :],
                            start=(dt == 0), stop=(dt == DT - 1))
                    vt = vo[sc * 4 + st]
                    nc.vector.tensor_copy(
                        vt[:, :, 0:64],
                        pv.rearrange("p (h j) -> p h j", h=HPC))
                    if need_round:
                        # memset on an f32r tile is ISA-invalid; copy from an
                        # f32 ones tile instead (copy rounds to f32r)
                        nc.vector.tensor_copy(
                            vt[:, :, 64:65],
                            ones_sb.rearrange("p (h o) -> p h o", o=1))
                    else:
                        nc.vector.memset(vt[:, :, 64:65], 1.0)

                # Q/K: out = (W row-slice) @ x^T -> [j, s] + RoPE
                for wname, dst in (("wq", QT), ("wk", KTt)):
                    for jt in range(4):
                        ps = paps.tile([128, SC], F32, tag=f"qkps{tag}", name=f"qkps{tag}")
                        jl = slice(jt * 128, (jt + 1) * 128)
                        for dt in range(DT):
                            nc.tensor.matmul(
                                ps, w_sb[wname][:, dt, jl], xc[:, dt, :],
                                start=(dt == 0), stop=(dt == DT - 1))
                        qs = pat.tile([128, SC], F32, tag=f"ropes{tag}", name=f"ropes{tag}")
                        nc.vector.stream_shuffle(qs, ps, _PAIR_SWAP)
                        qc_t = pat.tile([128, SC], F32, tag=f"ropec{tag}", name=f"ropec{tag}")
                        nc.vector.tensor_mul(qc_t, ps, cos_sb[:, ssl])
                        nc.vector.tensor_mul(qs, qs, sin_sb[:, ssl])
                        nc.vector.tensor_add(dst[jt][:, ssl], qc_t, qs)

        # ---- phase B: attention + output projection ---------------------
        with tc.tile_pool(name=f"pB{tag}", bufs=1) as pb, \
             tc.tile_pool(name=f"pBe{tag}", bufs=3) as pbe, \
             tc.tile_pool(name=f"pBt{tag}", bufs=2) as pbt, \
             tc.tile_pool(name=f"pBo{tag}", bufs=2) as pbo, \
             tc.tile_pool(name=f"pBps{tag}", bufs=1, space="PSUM") as pbps, \
             tc.tile_pool(name=f"pBps2{tag}", bufs=2, space="PSUM") as pbps2:
            if need_round:
                wostg = pbt.tile([128, 4, D], F32, tag=f"wostg{tag}", name=f"wostg{tag}", bufs=1)
                nc.sync.dma_start(
                    out=wostg, in_=wo.rearrange("(hp p) m -> p hp m", p=128))
                wo_sb = pb.tile([128, 4, D], mmdt, tag=f"wo{tag}", name=f"wo{tag}")
                nc.vector.tensor_copy(wo_sb, wostg)
            else:
                wo_sb = pb.tile([128, 4, D], mmdt, tag=f"wo{tag}", name=f"wo{tag}")
                nc.sync.dma_start(
                    out=wo_sb, in_=wo.rearrange("(hp p) m -> p hp m", p=128))

            for qc in range(NSC):
                qsl = slice(qc * SC, (qc + 1) * SC)
                oTs = []
                for hp in range(4):
                    pva = pbps.tile([65, SC], F32, tag=f"pva{tag}", name=f"pva{tag}")
                    pvb = pbps.tile([65, SC], F32, tag=f"pvb{tag}", name=f"pvb{tag}")
                    nkt = 4 * qc + 4
                    for kt in range(nkt):
                        ksl = slice(kt * KT, (kt + 1) * KT)
                        sca = pbps2.tile([128, SC], F32, tag=f"sca{tag}", name=f"sca{tag}")
                        scb = pbps2.tile([128, SC], F32, tag=f"scb{tag}", name=f"scb{tag}")
                        nc.tensor.matmul(sca, KTt[hp][0:64, ksl],
                                         QT[hp][0:64, qsl],
                                         start=True, stop=True,
                                         tile_position=(0, 0))
                        nc.tensor.matmul(scb, KTt[hp][64:128, ksl],
                                         QT[hp][64:128, qsl],
                                         start=True, stop=True,
                                         tile_position=(64, 0))
                        d = kt - 4 * qc
                        if d >= 0:  # diagonal tile: causal mask pre-exp
                            nc.vector.tensor_add(sca, sca, mask_sb[:, d, :])
                            nc.vector.tensor_add(scb, scb, mask_sb[:, d, :])
                        ea = pbe.tile([128, SC], mmdt, tag=f"ea{tag}", name=f"ea{tag}")
                        eb = pbe.tile([128, SC], mmdt, tag=f"eb{tag}", name=f"eb{tag}")
                        nc.scalar.activation(ea, sca, EXP, scale=0.125)
                        nc.scalar.activation(eb, scb, EXP, scale=0.125)
                        h0, h1 = 2 * hp, 2 * hp + 1
                        nc.tensor.matmul(
                            pva, vo[kt][:, h0, :], ea,
                            start=(kt == 0), stop=(kt == nkt - 1))
                        nc.tensor.matmul(
                            pvb, vo[kt][:, h1, :], eb,
                            start=(kt == 0), stop=(kt == nkt - 1))
                    # normalize: oT[j, q] = pv[j, q] / denom[q].
                    # All DVE ops must be partition-aligned; the denom row
                    # lives at partition 64, so recip in place (64->64), DMA
                    # the row to partition 0, broadcast to 0:64, multiply at
                    # base 0, and DMA-relocate head B's rows to 64:128.
                    rc = pbt.tile([65, 2, SC], F32, tag=f"rc{tag}", name=f"rc{tag}", bufs=1)
                    nc.vector.reciprocal(rc[64:65, 0, :], pva[64:65, :])
                    nc.vector.reciprocal(rc[64:65, 1, :], pvb[64:65, :])
                    r0 = pbt.tile([1, 2, SC], F32, tag=f"r0{tag}", name=f"r0{tag}", bufs=1)
                    nc.sync.dma_start(out=r0, in_=rc[64:65, :, :])
                    bc = pbt.tile([64, 2, SC], F32, tag=f"bc{tag}", name=f"bc{tag}")
                    nc.gpsimd.partition_broadcast(bc[:, 0, :], r0[:, 0, :])
                    nc.gpsimd.partition_broadcast(bc[:, 1, :], r0[:, 1, :])
                    bcA = bc[:, 0, :]
                    bcB = bc[:, 1, :]
                    oT = pbo.tile([128, SC], mmdt, tag=f"oT{hp}{tag}", name=f"oT{hp}{tag}")
                    tmpB = pbt.tile([64, SC], mmdt, tag=f"tmpB{tag}", name=f"tmpB{tag}")
                    nc.vector.tensor_mul(oT[0:64, :], pva[0:64, :], bcA)
                    nc.vector.tensor_mul(tmpB, pvb[0:64, :], bcB)
                    nc.sync.dma_start(out=oT[64:128, :], in_=tmpB)
                    oTs.append(oT)

                for mt in range(8):
                    yps = pbps2.tile([128, SC], F32, tag=f"yps{tag}", name=f"yps{tag}")
                    ml = slice(mt * 128, (mt + 1) * 128)
                    for hp in range(4):
                        nc.tensor.matmul(yps, wo_sb[:, hp, ml], oTs[hp],
                                         start=(hp == 0), stop=(hp == 3))
                    ys = pbt.tile([128, SC], F32, tag=f"ys{tag}", name=f"ys{tag}")
                    nc.vector.tensor_copy(ys, yps)
                    nc.sync.dma_start(out=yT[ml, qsl], in_=ys)


_BUILT = {}


def build_nc(mmdt_name=MM_DTYPE, repeat=1):
    key = (mmdt_name, repeat)
    if key in _BUILT:
        return _BUILT[key]
    mmdt = {"f32": F32, "f32r": F32R}[mmdt_name]
    nc = bacc.Bacc("TRN2", target_bir_lowering=False, debug=False,
                   num_devices=N_CORES)
    dram = {
        "xT": nc.dram_tensor("xT", [D, S], F32, kind="ExternalInput").ap(),
        "wq": nc.dram_tensor("wq", [D, JC], F32, kind="ExternalInput").ap(),
        "wk": nc.dram_tensor("wk", [D, JC], F32, kind="ExternalInput").ap(),
        "wv": nc.dram_tensor("wv", [D, JC], F32, kind="ExternalInput").ap(),
        "wo": nc.dram_tensor("wo", [JC, D], F32, kind="ExternalInput").ap(),
        "cosE": nc.dram_tensor("cosE", [128, S], F32,
                               kind="ExternalInput").ap(),
        "sinE": nc.dram_tensor("sinE", [128, S], F32,
                               kind="ExternalInput").ap(),
        "maskneg": nc.dram_tensor("maskneg", [128, 4 * SC], F32,
                                  kind="ExternalInput").ap(),
        "yT": nc.dram_tensor("yT", [D, S], F32, kind="ExternalOutput").ap(),
    }
    with tile.TileContext(nc) as tc:
        for r in range(repeat):
            _emit(nc, tc, mmdt, dram, tag=f"r{r}" if repeat > 1 else "")
    nc.compile()
    _BUILT[key] = nc
    return nc


def _host_prep(x, pos_ids, Wq, Wk, Wv, Wo, cos, sin):
    """Build the 8 per-core input maps."""
    x = np.asarray(x, dtype=np.float32)
    pos_ids = np.asarray(pos_ids)
    cos = np.asarray(cos, dtype=np.float32)
    sin = np.asarray(sin, dtype=np.float32)
    freq_idx = np.tile(np.repeat(np.arange(DK // 2), 2), 2)  # [128]
    sign = np.where((np.arange(128) % 2) == 0, -1.0, 1.0).astype(np.float32)

    # causal mask tiles: mask[d][p, q] = 0 if q >= 128*d + p else -1e30
    p = np.arange(128)[:, None]
    q = np.arange(SC)[None, :]
    mask = np.concatenate(
        [np.where(q >= 128 * d + p, 0.0, -1e30).astype(np.float32)
         for d in range(4)], axis=1)  # [128, 2048]

    in_maps = []
    for c in range(N_CORES):
        b, g = c // 2, c % 2
        hs = slice(64 * HPC * g, 64 * HPC * g + JC)
        pos = pos_ids[b].astype(np.int64)
        cosT = cos[pos].T  # [32, S]
        sinT = sin[pos].T
        cosE = np.ascontiguousarray(cosT[freq_idx])           # [128, S]
        sinE = np.ascontiguousarray(sinT[freq_idx] * sign[:, None])
        in_maps.append({
            "xT": np.ascontiguousarray(x[b].T),
            "wq": np.ascontiguousarray(Wq[hs, :].T),
            "wk": np.ascontiguousarray(Wk[hs, :].T),
            "wv": np.ascontiguousarray(Wv[hs, :].T),
            "wo": np.ascontiguousarray(Wo[:, hs].T),
            "cosE": cosE,
            "sinE": sinE,
            "maskneg": mask,
        })
    return in_maps


def kernel(x, pos_ids, Wq, Wk, Wv, Wo, cos, sin):
    nc = build_nc()
    in_maps = _host_prep(x, pos_ids, Wq, Wk, Wv, Wo, cos, sin)
    res = run_bass_kernel_spmd(nc, in_maps, list(range(N_CORES)))
    out = np.empty((B, S, D), dtype=np.float32)
    for b in range(B):
        yT = res.results[2 * b]["yT"] + res.results[2 * b + 1]["yT"]
        out[b] = yT.T
    return out


if __name__ == "__main__":
    t0 = time.time()
    nc = build_nc()
    print(f"build+compile: {time.time()-t0:.1f}s", flush=True)
